# revision 52
# baseline (speedup 1.0000x reference)
"""Trainium2 Bass kernel for prefix-causal sparse attention (GPT-style block).

Reference computation (per batch element b):
    qkv = x @ W_attn + b_attn                     # [S, 3D]
    q, k, v = split(qkv); heads H=8, hd=128
    s = q @ k.T / sqrt(hd)  with prefix-causal mask (rows<77 attend cols<77,
        rows>=77 causal) as multiplicative 0/1 post-exp (equivalent to the
        reference's -10000 replacement), + attention_mask bias
    a = softmax(s) @ v; out = merge_heads(a) @ W_proj + b_proj

Distribution: pure data parallelism, batch B=8 over 8 NeuronCores (one batch
element per core). No collectives.

Per-core layout strategy (all matmuls bf16, 1 cyc/row + FWL weight loads):
  - x^T [D, S] pre-transposed on host, W_attn host-permuted into contiguous
    [group, p, kt, m] tiles, biases/attention_mask host-prearranged columns
  - Q^T/K^T per head [hd=128, S] computed directly in transposed layout
    (lhsT = W_attn columns, rhs = x^T); first W group k-outer across 8 live
    PSUM banks so matmuls start as soon as each DMA k-slice lands
  - scores computed TRANSPOSED [k, q]: softmax denominator via a ones[128,128]
    matmul (broadcast across partitions for free), prefix/causal masks as
    multiplicative 0/1 bf16 tiles post-exp (== reference's -10000 replace),
    attention_mask folded in as the exp's per-partition bias; no max
    subtraction needed (scores are O(1)); P^T feeds the AV matmul directly
    (no P transposes anywhere)
  - A^T [D, S] = unnormalized AV accumulated in PSUM, normalized by
    reciprocal_approx_fast(l) on VectorE; head-pipelined emission hides the
    exp latency; head 0's scores are pre-emitted before the V GEMM
  - proj consumes A^T as lhsT -> natural [S, D] output, interleaved per
    q-half with the other half's attention
"""

import sys

import numpy as np

if "/opt/trn_rl_repo" not in sys.path:
    sys.path.insert(0, "/opt/trn_rl_repo")

B, S, D, H, HD = 8, 1024, 1024, 8, 128
PREFIX_LEN = 77
N_CORES = 8

_CACHE = {}


def _build():
    import ml_dtypes
    import concourse.bass as bass  # noqa
    import concourse.mybir as mybir
    import concourse.tile as tile
    from concourse import bacc
    from contextlib import ExitStack

    FP = mybir.dt.float32
    FR = mybir.dt.float32r
    BF = mybir.dt.bfloat16
    AF = mybir.ActivationFunctionType

    INV_SQRT_HD = 1.0 / float(np.sqrt(HD))
    NKT = S // 128  # 8 k-tiles per head-row of scores

    nc = bacc.Bacc(None)

    # x arrives pre-transposed [D, S]; W_attn pre-permuted into contiguous
    # [group, p, kt, m] streaming layout (host-side prep in kernel()).
    xT_ext = nc.declare_dram_parameter("xT", [D, S], BF, isOutput=False)
    amcol_ext = nc.declare_dram_parameter("amcol", [128, 8], FP, isOutput=False)
    bcols_ext = nc.declare_dram_parameter("bcols", [128, 24], FP, isOutput=False)
    wa_ext = nc.declare_dram_parameter("W_attn", [6, 128, 8, 512], BF, isOutput=False)
    ba_ext = nc.declare_dram_parameter("b_attn", [3 * D], FP, isOutput=False)
    wp_ext = nc.declare_dram_parameter("W_proj", [D, D], BF, isOutput=False)
    bp_ext = nc.declare_dram_parameter("b_proj", [D], FP, isOutput=False)
    out_ext = nc.declare_dram_parameter("out", [S, D], FP, isOutput=True)

    # Inline constants
    ones_d = nc.inline_tensor(
        np.ones((128, 128), dtype=ml_dtypes.bfloat16), name="ones_bf"
    )
    # Masks in scores-TRANSPOSED layout: element [k, q] = 1 if q attends k.
    kk = np.arange(128)[:, None]
    qq = np.arange(128)[None, :]
    diag_np = (qq >= kk).astype(np.float32)  # causal within diagonal tile
    m0_np = np.where(qq < PREFIX_LEN, (kk < PREFIX_LEN), (kk <= qq)).astype(np.float32)
    maskdiag_d = nc.inline_tensor(diag_np.astype(ml_dtypes.bfloat16), name="maskdiag")
    mask0_d = nc.inline_tensor(m0_np.astype(ml_dtypes.bfloat16), name="mask0")

    with ExitStack() as stk:
        tc = stk.enter_context(tile.TileContext(nc))

        cpool = stk.enter_context(tc.tile_pool(name="consts", bufs=1))
        ones_bf = cpool.tile([128, 128], BF)
        nc.gpsimd.dma_start(ones_bf[:], ones_d[:])
        maskdiag = cpool.tile([128, 128], BF)
        nc.gpsimd.dma_start(maskdiag[:], maskdiag_d[:])
        mask0 = cpool.tile([128, 128], BF)
        nc.gpsimd.dma_start(mask0[:], mask0_d[:])
        # 8-bank psum pool for phases 0/1; closed before the attention pools.
        psum1_cm = tc.tile_pool(name="psum1", bufs=1, space="PSUM")
        psum1 = psum1_cm.__enter__()
        # bias/mask columns arrive pre-arranged from the host
        b_cols = cpool.tile([128, 24], FP)
        nc.gpsimd.dma_start(b_cols[:], bcols_ext[:])
        am_col = cpool.tile([128, 8], FP)
        nc.gpsimd.dma_start(am_col[:], amcol_ext[:])
        bq_scaled = cpool.tile([128, 8], FP)
        nc.scalar.mul(bq_scaled[:], b_cols[:, 0:8], INV_SQRT_HD)
        # broadcast bias rows for V and proj epilogues
        bv_row = cpool.tile([1, D], FP)
        nc.gpsimd.dma_start(bv_row[:], ba_ext[2 * D : 3 * D][None, :])
        bvb = cpool.tile([128, D], FP)
        nc.gpsimd.partition_broadcast(bvb[:], bv_row[:])

        # Persistent activation storage
        qkT_pool = stk.enter_context(tc.tile_pool(name="qkT", bufs=1))
        qkT = [qkT_pool.tile([128, S], BF, tag=f"qkT{m}", name=f"qkT{m}") for m in range(16)]
        v_pool = stk.enter_context(tc.tile_pool(name="vpool", bufs=1))
        v_bf = [v_pool.tile([128, D], BF, tag=f"v{st}", name=f"v{st}") for st in range(8)]
        pT_pool = stk.enter_context(tc.tile_pool(name="pTpool", bufs=18))
        psb_pool = stk.enter_context(tc.tile_pool(name="psbpool", bufs=3))

        def emit_scores_any(c, h, pspool, pstag, psbufs):
            # score matmuls + exp + diag masks for head h / q-half c
            last_ = min(NKT - 1, 4 * c + 3)
            pts = []
            for kt in range(last_ + 1):
                qlo = max(kt * 128, c * 512)
                w = (c + 1) * 512 - qlo
                pT = pT_pool.tile([128, 512], BF, tag="pT", name=f"pT{c}_{h}_{kt}")
                ps_s = pspool.tile([128, 512], FP, tag=pstag, bufs=psbufs, name="ps_s")
                nc.tensor.matmul(
                    ps_s[:, :w],
                    qkT[8 + h][:, kt * 128 : (kt + 1) * 128],
                    qkT[h][:, qlo : qlo + w],
                    start=True,
                    stop=True,
                )
                nc.scalar.activation(
                    pT[:, :w], ps_s[:, :w], AF.Exp, bias=am_col[:, kt : kt + 1], scale=1.0
                )
                if kt * 128 >= c * 512:  # diagonal block at piece start
                    nc.vector.tensor_mul(
                        pT[:, 0:128], pT[:, 0:128], mask0[:] if kt == 0 else maskdiag[:]
                    )
                pts.append((pT, qlo, w))
            return pts

        # ---- Phase 0: load x^T (pre-transposed on host) ----
        with tc.tile_pool(name="xTpool", bufs=1) as xT_pool:
            xT = [xT_pool.tile([128, S], BF, tag=f"xT{d}", name=f"xT{d}") for d in range(8)]
            # split halves, interleaved per d so each k-slice's full row
            # becomes available progressively for the k-outer first group
            for d in range(8):
                for half in range(2):
                    nc.sync.dma_start(
                        xT[d][:, half * 512 : (half + 1) * 512],
                        xT_ext[d * 128 : (d + 1) * 128, half * 512 : (half + 1) * 512],
                    )
            # W streaming pool shared by phase 1a/1b so later groups prefetch
            # during earlier compute. W DMAs dispatched from the scalar engine
            # queue so they don't serialize behind x loads on sync.
            with tc.tile_pool(name="wstream", bufs=2) as w_pool:
                wtiles = []
                for g in range(6):  # 6 groups of 512 cols covering all 3072
                    w = w_pool.tile([128, 8, 512], BF, tag="w", name=f"w{g}")
                    if g == 0:
                        # per-k slices so the first matmuls unblock ASAP
                        for k in range(8):
                            nc.scalar.dma_start(
                                w[:, k : k + 1, :], wa_ext[g, :, k : k + 1, :]
                            )
                    else:
                        nc.scalar.dma_start(w[:], wa_ext[g])
                    wtiles.append(w)

                # ---- Phase 1a: Q^T, K^T (m-tiles 0..15 of qkv^T) ----
                # Dedicated 8-bank psum pool (released before the attention
                # pools open) so all 8 accumulators of a W group stay live.
                if True:

                    def qk_epilogue(m, n, ps):
                        if m < 8:  # Q: scale by 1/sqrt(hd), bias pre-scaled
                            nc.scalar.activation(
                                qkT[m][:, n * 512 : (n + 1) * 512],
                                ps[:],
                                AF.Identity,
                                bias=bq_scaled[:, m : m + 1],
                                scale=INV_SQRT_HD,
                            )
                        else:  # K: plain bias
                            nc.scalar.activation(
                                qkT[m][:, n * 512 : (n + 1) * 512],
                                ps[:],
                                AF.Identity,
                                bias=b_cols[:, m : m + 1],
                                scale=1.0,
                            )

                    # group 0: k-outer so matmuls start as soon as each
                    # k-slice of W / xT lands (the front is DMA-paced)
                    g0_ps = [
                        psum1.tile([128, 512], FP, tag="p1", bufs=8, name=f"g0ps{i}")
                        for i in range(8)
                    ]
                    for k in range(8):
                        for mloc in range(4):
                            for n in range(2):
                                nc.tensor.matmul(
                                    g0_ps[mloc * 2 + n][:],
                                    wtiles[0][:, k, mloc * 128 : (mloc + 1) * 128],
                                    xT[k][:, n * 512 : (n + 1) * 512],
                                    start=(k == 0),
                                    stop=(k == 7),
                                )
                    for mloc in range(4):
                        for n in range(2):
                            qk_epilogue(mloc, n, g0_ps[mloc * 2 + n])

                    for g in range(1, 4):
                        wqk = wtiles[g]
                        for mloc in range(4):
                            m = g * 4 + mloc
                            for n in range(2):
                                ps = psum1.tile(
                                    [128, 512], FP, tag="p1", bufs=8, name="qk_ps"
                                )
                                for k in range(8):
                                    nc.tensor.matmul(
                                        ps[:],
                                        wqk[:, k, mloc * 128 : (mloc + 1) * 128],
                                        xT[k][:, n * 512 : (n + 1) * 512],
                                        start=(k == 0),
                                        stop=(k == 7),
                                    )
                                qk_epilogue(m, n, ps)

                    # prime the attention pipeline: head 0 / half 0 scores
                    # run here (psum1 slots) so their exps complete during V
                    pre_pts = emit_scores_any(0, 0, psum1, "p1", 8)

                    # ---- Phase 1b: V natural [S, D] in bf16 ----
                    for n in range(2):
                        wv = wtiles[4 + n]
                        for st in range(8):
                            ps = psum1.tile(
                                [128, 512], FP, tag="p1", bufs=8, name="v_ps"
                            )
                            for k in range(8):
                                nc.tensor.matmul(
                                    ps[:],
                                    xT[k][:, st * 128 : (st + 1) * 128],
                                    wv[:, k, :],
                                    start=(k == 0),
                                    stop=(k == 7),
                                )
                            nc.vector.tensor_add(
                                v_bf[st][:, n * 512 : (n + 1) * 512],
                                ps[:],
                                bvb[:, n * 512 : (n + 1) * 512],
                            )

        # xT released here.
        psum1_cm.__exit__(None, None, None)
        psum2 = stk.enter_context(tc.tile_pool(name="psum2", bufs=1, space="PSUM"))
        aT_pool = stk.enter_context(tc.tile_pool(name="aTpool", bufs=1))
        aT = [aT_pool.tile([128, S], BF, tag=f"aT{d}", name=f"aT{d}") for d in range(8)]

        # ---- Phase 3 weights prefetch + Phase 2 ----
        with tc.tile_pool(name="wp", bufs=1) as wp_pool:
            wp = [wp_pool.tile([128, D], BF, tag=f"wp{d}", name=f"wp{d}") for d in range(8)]
            for d in range(8):
                nc.sync.dma_start(wp[d][:], wp_ext[d * 128 : (d + 1) * 128, :])

            # ---- Phase 2 (attention) interleaved with Phase 3 (proj) ----
            # q-half c outer, heads inner; after half c's attention, proj rows
            # st in [4c, 4c+4) only need aT columns of that half, so the proj
            # epilogue overlaps the next half's attention.
            with (
                tc.tile_pool(name="recip", bufs=4) as recip_pool,
                tc.tile_pool(name="osb", bufs=6) as osb_pool,
            ):
                for c in range(2):  # q halves [0,512), [512,1024)
                    last = min(NKT - 1, 4 * c + 3)

                    def emit_lav(h, pts):
                        ps_l = psum2.tile([128, 512], FP, tag="l", bufs=2, name="ps_l")
                        ps_a = psum2.tile([128, 512], FP, tag="a", bufs=2, name="ps_a")
                        # P_sum = elementwise bf16 sum of the pT pieces (base
                        # copy on ACT, adds on DVE), so the softmax denominator
                        # needs ONE ones-matmul instead of one per k-tile
                        psb = psb_pool.tile([128, 512], BF, tag="psb", name="psb")
                        for kt, (pT, qlo, w) in enumerate(pts):
                            off = qlo - c * 512
                            nc.tensor.matmul(
                                ps_a[:, off : off + w],
                                v_bf[kt][:, h * 128 : (h + 1) * 128],
                                pT[:, :w],
                                start=(kt == 0),
                                stop=(kt == last),
                            )
                            if kt == 0:  # kt0 piece always spans the full half
                                nc.scalar.copy(psb[:], pT[:])
                            else:
                                nc.vector.tensor_add(
                                    psb[:, off : off + w],
                                    psb[:, off : off + w],
                                    pT[:, 0:w],
                                )
                        nc.tensor.matmul(
                            ps_l[:], ones_bf[:], psb[:], start=True, stop=True
                        )
                        recip_l = recip_pool.tile([128, 512], FP, tag="recip")
                        nc.vector.reciprocal_approx_fast(recip_l[:], ps_l[:])
                        nc.vector.tensor_mul(
                            aT[h][:, c * 512 : (c + 1) * 512], ps_a[:], recip_l[:]
                        )

                    # software-pipelined emission: scores(h+1) land on PE
                    # before l/av(h), hiding the exp latency at head starts.
                    # For c=0, head 0's scores were pre-emitted before V.
                    prev = pre_pts if c == 0 else None
                    for h in range(1 if c == 0 else 0, 8):
                        pts = emit_scores_any(c, h, psum2, "mm", 4)
                        if prev is not None:
                            emit_lav(h - 1, prev)
                        prev = pts
                    emit_lav(7, prev)

                    # proj rows for this half: out = A @ W_proj + b_proj
                    for st in range(4 * c, 4 * c + 4):
                        for n in range(2):
                            ps = psum2.tile([128, 512], FP, tag="mm", bufs=4, name="o_ps")
                            for d in range(8):
                                nc.tensor.matmul(
                                    ps[:],
                                    aT[d][:, st * 128 : (st + 1) * 128],
                                    wp[d][:, n * 512 : (n + 1) * 512],
                                    start=(d == 0),
                                    stop=(d == 7),
                                )
                            out_sb = osb_pool.tile([128, 512], FP, tag="osb")
                            nc.scalar.copy(out_sb[:], ps[:])
                            dma_eng = nc.sync if n == 0 else nc.scalar
                            dma_eng.dma_start(
                                out_ext[
                                    st * 128 : (st + 1) * 128,
                                    n * 512 : (n + 1) * 512,
                                ],
                                out_sb[:],
                            )

    nc.compile()
    return nc


def _get_nc():
    if "nc" not in _CACHE:
        _CACHE["nc"] = _build()
    return _CACHE["nc"]


def kernel(x, attention_mask, W_attn, b_attn, W_proj, b_proj, **kwargs):
    from concourse.bass_utils import run_bass_kernel_spmd

    nc = _get_nc()
    x = np.ascontiguousarray(np.asarray(x, dtype=np.float32))
    am = np.ascontiguousarray(
        np.asarray(attention_mask, dtype=np.float32).reshape(B, S)
    )
    W_attn = np.ascontiguousarray(np.asarray(W_attn, dtype=np.float32))
    b_attn = np.ascontiguousarray(np.asarray(b_attn, dtype=np.float32))
    W_proj = np.ascontiguousarray(np.asarray(W_proj, dtype=np.float32))
    b_proj = np.ascontiguousarray(np.asarray(b_proj, dtype=np.float32))

    import ml_dtypes

    W_attn_bf = np.ascontiguousarray(
        W_attn.astype(ml_dtypes.bfloat16)
        .reshape(8, 128, 6, 512)
        .transpose(2, 1, 0, 3)
    )
    W_proj_bf = np.ascontiguousarray(W_proj.astype(ml_dtypes.bfloat16))
    b_cols = np.ascontiguousarray(b_attn.reshape(24, 128).T)
    in_maps = [
        {
            "xT": np.ascontiguousarray(x[b].T.astype(ml_dtypes.bfloat16)),
            "amcol": np.ascontiguousarray(am[b].reshape(8, 128).T),
            "bcols": b_cols,
            "W_attn": W_attn_bf,
            "b_attn": b_attn,
            "W_proj": W_proj_bf,
            "b_proj": b_proj,
        }
        for b in range(N_CORES)
    ]
    import time

    res = None
    for attempt in range(3):
        try:
            res = run_bass_kernel_spmd(nc, in_maps, list(range(N_CORES)))
            break
        except Exception:
            # transient device hiccups (e.g. NRT exec-unit unrecoverable)
            # usually clear after a short wait
            if attempt == 2:
                raise
            time.sleep(20 * (attempt + 1))
    out = np.stack([res.results[b]["out"] for b in range(N_CORES)], axis=0)
    # b_proj is a constant row: applied on host (frees the on-device epilogue)
    out = out + b_proj[None, None, :]
    return out.astype(np.float32)


# revision 53
# speedup vs baseline: 1.0639x; 1.0639x over previous
"""Trainium2 Bass kernel for prefix-causal sparse attention (GPT-style block).

Reference computation (per batch element b):
    qkv = x @ W_attn + b_attn                     # [S, 3D]
    q, k, v = split(qkv); heads H=8, hd=128
    s = q @ k.T / sqrt(hd)  with prefix-causal mask (rows<77 attend cols<77,
        rows>=77 causal) as multiplicative 0/1 post-exp (equivalent to the
        reference's -10000 replacement), + attention_mask bias
    a = softmax(s) @ v; out = merge_heads(a) @ W_proj + b_proj

Distribution: pure data parallelism, batch B=8 over 8 NeuronCores (one batch
element per core). No collectives.

Per-core layout strategy (all matmuls bf16, 1 cyc/row + FWL weight loads):
  - x^T [D, S] pre-transposed on host, W_attn host-permuted into contiguous
    [group, p, kt, m] tiles, biases/attention_mask host-prearranged columns
  - Q^T/K^T per head [hd=128, S] computed directly in transposed layout
    (lhsT = W_attn columns, rhs = x^T); first W group k-outer across 8 live
    PSUM banks so matmuls start as soon as each DMA k-slice lands
  - scores computed TRANSPOSED [k, q]: softmax denominator via a ones[128,128]
    matmul (broadcast across partitions for free), prefix/causal masks as
    multiplicative 0/1 bf16 tiles post-exp (== reference's -10000 replace),
    attention_mask folded in as the exp's per-partition bias; no max
    subtraction needed (scores are O(1)); P^T feeds the AV matmul directly
    (no P transposes anywhere)
  - A^T [D, S] = unnormalized AV accumulated in PSUM, normalized by
    reciprocal_approx_fast(l) on VectorE; head-pipelined emission hides the
    exp latency; head 0's scores are pre-emitted before the V GEMM
  - proj consumes A^T as lhsT -> natural [S, D] output, interleaved per
    q-half with the other half's attention
"""

import sys

import numpy as np

if "/opt/trn_rl_repo" not in sys.path:
    sys.path.insert(0, "/opt/trn_rl_repo")

B, S, D, H, HD = 8, 1024, 1024, 8, 128
PREFIX_LEN = 77
N_CORES = 8

_CACHE = {}


def _build():
    import ml_dtypes
    import concourse.bass as bass  # noqa
    import concourse.mybir as mybir
    import concourse.tile as tile
    from concourse import bacc
    from contextlib import ExitStack

    FP = mybir.dt.float32
    FR = mybir.dt.float32r
    BF = mybir.dt.bfloat16
    AF = mybir.ActivationFunctionType

    INV_SQRT_HD = 1.0 / float(np.sqrt(HD))
    NKT = S // 128  # 8 k-tiles per head-row of scores

    nc = bacc.Bacc(None)

    # x arrives pre-transposed [D, S]; W_attn pre-permuted into contiguous
    # [group, p, kt, m] streaming layout (host-side prep in kernel()).
    xT_ext = nc.declare_dram_parameter("xT", [D, S], BF, isOutput=False)
    amcol_ext = nc.declare_dram_parameter("amcol", [128, 8], FP, isOutput=False)
    bcols_ext = nc.declare_dram_parameter("bcols", [128, 24], FP, isOutput=False)
    wa_ext = nc.declare_dram_parameter("W_attn", [6, 128, 8, 512], BF, isOutput=False)
    ba_ext = nc.declare_dram_parameter("b_attn", [3 * D], FP, isOutput=False)
    wp_ext = nc.declare_dram_parameter("W_proj", [D, D], BF, isOutput=False)
    bp_ext = nc.declare_dram_parameter("b_proj", [D], FP, isOutput=False)
    out_ext = nc.declare_dram_parameter("out", [S, D], FP, isOutput=True)

    # Inline constants
    ones_d = nc.inline_tensor(
        np.ones((128, 128), dtype=ml_dtypes.bfloat16), name="ones_bf"
    )
    # Masks in scores-TRANSPOSED layout: element [k, q] = 1 if q attends k.
    kk = np.arange(128)[:, None]
    qq = np.arange(128)[None, :]
    diag_np = (qq >= kk).astype(np.float32)  # causal within diagonal tile
    m0_np = np.where(qq < PREFIX_LEN, (kk < PREFIX_LEN), (kk <= qq)).astype(np.float32)
    maskdiag_d = nc.inline_tensor(diag_np.astype(ml_dtypes.bfloat16), name="maskdiag")
    mask0_d = nc.inline_tensor(m0_np.astype(ml_dtypes.bfloat16), name="mask0")

    with ExitStack() as stk:
        tc = stk.enter_context(tile.TileContext(nc))

        cpool = stk.enter_context(tc.tile_pool(name="consts", bufs=1))
        ones_bf = cpool.tile([128, 128], BF)
        nc.gpsimd.dma_start(ones_bf[:], ones_d[:])
        maskdiag = cpool.tile([128, 128], BF)
        nc.gpsimd.dma_start(maskdiag[:], maskdiag_d[:])
        mask0 = cpool.tile([128, 128], BF)
        nc.gpsimd.dma_start(mask0[:], mask0_d[:])
        # 8-bank psum pool for phases 0/1; closed before the attention pools.
        psum1_cm = tc.tile_pool(name="psum1", bufs=1, space="PSUM")
        psum1 = psum1_cm.__enter__()
        # bias/mask columns arrive pre-arranged from the host
        b_cols = cpool.tile([128, 24], FP)
        nc.gpsimd.dma_start(b_cols[:], bcols_ext[:])
        am_col = cpool.tile([128, 8], FP)
        nc.gpsimd.dma_start(am_col[:], amcol_ext[:])
        bq_scaled = cpool.tile([128, 8], FP)
        nc.scalar.mul(bq_scaled[:], b_cols[:, 0:8], INV_SQRT_HD)
        # broadcast bias rows for V and proj epilogues
        bv_row = cpool.tile([1, D], FP)
        nc.gpsimd.dma_start(bv_row[:], ba_ext[2 * D : 3 * D][None, :])
        bvb = cpool.tile([128, D], FP)
        nc.gpsimd.partition_broadcast(bvb[:], bv_row[:])

        # Persistent activation storage
        qkT_pool = stk.enter_context(tc.tile_pool(name="qkT", bufs=1))
        qkT = [qkT_pool.tile([128, S], BF, tag=f"qkT{m}", name=f"qkT{m}") for m in range(16)]
        v_pool = stk.enter_context(tc.tile_pool(name="vpool", bufs=1))
        v_bf = [v_pool.tile([128, D], BF, tag=f"v{st}", name=f"v{st}") for st in range(8)]
        pT_pool = stk.enter_context(tc.tile_pool(name="pTpool", bufs=18))

        def emit_scores_any(c, h, pspool, pstag, psbufs):
            # score matmuls + exp + diag masks for head h / q-half c
            last_ = min(NKT - 1, 4 * c + 3)
            pts = []
            for kt in range(last_ + 1):
                qlo = max(kt * 128, c * 512)
                w = (c + 1) * 512 - qlo
                pT = pT_pool.tile([128, 512], BF, tag="pT", name=f"pT{c}_{h}_{kt}")
                ps_s = pspool.tile([128, 512], FP, tag=pstag, bufs=psbufs, name="ps_s")
                nc.tensor.matmul(
                    ps_s[:, :w],
                    qkT[8 + h][:, kt * 128 : (kt + 1) * 128],
                    qkT[h][:, qlo : qlo + w],
                    start=True,
                    stop=True,
                )
                nc.scalar.activation(
                    pT[:, :w], ps_s[:, :w], AF.Exp, bias=am_col[:, kt : kt + 1], scale=1.0
                )
                if kt * 128 >= c * 512:  # diagonal block at piece start
                    nc.vector.tensor_mul(
                        pT[:, 0:128], pT[:, 0:128], mask0[:] if kt == 0 else maskdiag[:]
                    )
                pts.append((pT, qlo, w))
            return pts

        # ---- Phase 0: load x^T (pre-transposed on host) ----
        with tc.tile_pool(name="xTpool", bufs=1) as xT_pool:
            xT = [xT_pool.tile([128, S], BF, tag=f"xT{d}", name=f"xT{d}") for d in range(8)]
            # split halves, interleaved per d so each k-slice's full row
            # becomes available progressively for the k-outer first group
            for d in range(8):
                for half in range(2):
                    nc.sync.dma_start(
                        xT[d][:, half * 512 : (half + 1) * 512],
                        xT_ext[d * 128 : (d + 1) * 128, half * 512 : (half + 1) * 512],
                    )
            # W streaming pool shared by phase 1a/1b so later groups prefetch
            # during earlier compute. W DMAs dispatched from the scalar engine
            # queue so they don't serialize behind x loads on sync.
            with tc.tile_pool(name="wstream", bufs=2) as w_pool:
                wtiles = []
                for g in range(6):  # 6 groups of 512 cols covering all 3072
                    w = w_pool.tile([128, 8, 512], BF, tag="w", name=f"w{g}")
                    if g == 0:
                        # per-k slices so the first matmuls unblock ASAP
                        for k in range(8):
                            nc.scalar.dma_start(
                                w[:, k : k + 1, :], wa_ext[g, :, k : k + 1, :]
                            )
                    else:
                        nc.scalar.dma_start(w[:], wa_ext[g])
                    wtiles.append(w)

                # ---- Phase 1a: Q^T, K^T (m-tiles 0..15 of qkv^T) ----
                # Dedicated 8-bank psum pool (released before the attention
                # pools open) so all 8 accumulators of a W group stay live.
                if True:

                    def qk_epilogue(m, n, ps):
                        if m < 8:  # Q: scale by 1/sqrt(hd), bias pre-scaled
                            nc.scalar.activation(
                                qkT[m][:, n * 512 : (n + 1) * 512],
                                ps[:],
                                AF.Identity,
                                bias=bq_scaled[:, m : m + 1],
                                scale=INV_SQRT_HD,
                            )
                        else:  # K: plain bias
                            nc.scalar.activation(
                                qkT[m][:, n * 512 : (n + 1) * 512],
                                ps[:],
                                AF.Identity,
                                bias=b_cols[:, m : m + 1],
                                scale=1.0,
                            )

                    # group 0: k-outer so matmuls start as soon as each
                    # k-slice of W / xT lands (the front is DMA-paced)
                    g0_ps = [
                        psum1.tile([128, 512], FP, tag="p1", bufs=8, name=f"g0ps{i}")
                        for i in range(8)
                    ]
                    for k in range(8):
                        for mloc in range(4):
                            for n in range(2):
                                nc.tensor.matmul(
                                    g0_ps[mloc * 2 + n][:],
                                    wtiles[0][:, k, mloc * 128 : (mloc + 1) * 128],
                                    xT[k][:, n * 512 : (n + 1) * 512],
                                    start=(k == 0),
                                    stop=(k == 7),
                                )
                    for mloc in range(4):
                        for n in range(2):
                            qk_epilogue(mloc, n, g0_ps[mloc * 2 + n])

                    for g in range(1, 4):
                        wqk = wtiles[g]
                        for mloc in range(4):
                            m = g * 4 + mloc
                            for n in range(2):
                                ps = psum1.tile(
                                    [128, 512], FP, tag="p1", bufs=8, name="qk_ps"
                                )
                                for k in range(8):
                                    nc.tensor.matmul(
                                        ps[:],
                                        wqk[:, k, mloc * 128 : (mloc + 1) * 128],
                                        xT[k][:, n * 512 : (n + 1) * 512],
                                        start=(k == 0),
                                        stop=(k == 7),
                                    )
                                qk_epilogue(m, n, ps)

                    # prime the attention pipeline: head 0 / half 0 scores
                    # run here (psum1 slots) so their exps complete during V
                    pre_pts = emit_scores_any(0, 0, psum1, "p1", 8)

                    # ---- Phase 1b: V natural [S, D] in bf16 ----
                    for n in range(2):
                        wv = wtiles[4 + n]
                        for st in range(8):
                            ps = psum1.tile(
                                [128, 512], FP, tag="p1", bufs=8, name="v_ps"
                            )
                            for k in range(8):
                                nc.tensor.matmul(
                                    ps[:],
                                    xT[k][:, st * 128 : (st + 1) * 128],
                                    wv[:, k, :],
                                    start=(k == 0),
                                    stop=(k == 7),
                                )
                            nc.vector.tensor_add(
                                v_bf[st][:, n * 512 : (n + 1) * 512],
                                ps[:],
                                bvb[:, n * 512 : (n + 1) * 512],
                            )

        # xT released here.
        psum1_cm.__exit__(None, None, None)
        psum2 = stk.enter_context(tc.tile_pool(name="psum2", bufs=1, space="PSUM"))
        aT_pool = stk.enter_context(tc.tile_pool(name="aTpool", bufs=1))
        aT = [aT_pool.tile([128, S], BF, tag=f"aT{d}", name=f"aT{d}") for d in range(8)]

        # ---- Phase 3 weights prefetch + Phase 2 ----
        with tc.tile_pool(name="wp", bufs=1) as wp_pool:
            wp = [wp_pool.tile([128, D], BF, tag=f"wp{d}", name=f"wp{d}") for d in range(8)]
            for d in range(8):
                nc.sync.dma_start(wp[d][:], wp_ext[d * 128 : (d + 1) * 128, :])

            # ---- Phase 2 (attention) interleaved with Phase 3 (proj) ----
            # q-half c outer, heads inner; after half c's attention, proj rows
            # st in [4c, 4c+4) only need aT columns of that half, so the proj
            # epilogue overlaps the next half's attention.
            with (
                tc.tile_pool(name="recip", bufs=4) as recip_pool,
                tc.tile_pool(name="osb", bufs=6) as osb_pool,
            ):
                for c in range(2):  # q halves [0,512), [512,1024)
                    last = min(NKT - 1, 4 * c + 3)

                    def emit_lav(h, pts):
                        ps_l = psum2.tile([128, 512], FP, tag="l", bufs=2, name="ps_l")
                        ps_a = psum2.tile([128, 512], FP, tag="a", bufs=2, name="ps_a")
                        for kt, (pT, qlo, w) in enumerate(pts):
                            off = qlo - c * 512
                            nc.tensor.matmul(
                                ps_l[:, off : off + w],
                                ones_bf[:],
                                pT[:, :w],
                                start=(kt == 0),
                                stop=(kt == last),
                            )
                            nc.tensor.matmul(
                                ps_a[:, off : off + w],
                                v_bf[kt][:, h * 128 : (h + 1) * 128],
                                pT[:, :w],
                                start=(kt == 0),
                                stop=(kt == last),
                            )
                        recip_l = recip_pool.tile([128, 512], FP, tag="recip")
                        nc.vector.reciprocal_approx_fast(recip_l[:], ps_l[:])
                        nc.vector.tensor_mul(
                            aT[h][:, c * 512 : (c + 1) * 512], ps_a[:], recip_l[:]
                        )

                    # software-pipelined emission: scores(h+1) land on PE
                    # before l/av(h), hiding the exp latency at head starts.
                    # For c=0, head 0's scores were pre-emitted before V.
                    prev = pre_pts if c == 0 else None
                    for h in range(1 if c == 0 else 0, 8):
                        pts = emit_scores_any(c, h, psum2, "mm", 4)
                        if prev is not None:
                            emit_lav(h - 1, prev)
                        prev = pts
                    emit_lav(7, prev)

                    # proj rows for this half: out = A @ W_proj + b_proj
                    for st in range(4 * c, 4 * c + 4):
                        for n in range(2):
                            ps = psum2.tile([128, 512], FP, tag="mm", bufs=4, name="o_ps")
                            for d in range(8):
                                nc.tensor.matmul(
                                    ps[:],
                                    aT[d][:, st * 128 : (st + 1) * 128],
                                    wp[d][:, n * 512 : (n + 1) * 512],
                                    start=(d == 0),
                                    stop=(d == 7),
                                )
                            out_sb = osb_pool.tile([128, 512], FP, tag="osb")
                            nc.scalar.copy(out_sb[:], ps[:])
                            dma_eng = nc.sync if n == 0 else nc.scalar
                            dma_eng.dma_start(
                                out_ext[
                                    st * 128 : (st + 1) * 128,
                                    n * 512 : (n + 1) * 512,
                                ],
                                out_sb[:],
                            )

    nc.compile()
    return nc


def _get_nc():
    if "nc" not in _CACHE:
        _CACHE["nc"] = _build()
    return _CACHE["nc"]


def kernel(x, attention_mask, W_attn, b_attn, W_proj, b_proj, **kwargs):
    from concourse.bass_utils import run_bass_kernel_spmd

    nc = _get_nc()
    x = np.ascontiguousarray(np.asarray(x, dtype=np.float32))
    am = np.ascontiguousarray(
        np.asarray(attention_mask, dtype=np.float32).reshape(B, S)
    )
    W_attn = np.ascontiguousarray(np.asarray(W_attn, dtype=np.float32))
    b_attn = np.ascontiguousarray(np.asarray(b_attn, dtype=np.float32))
    W_proj = np.ascontiguousarray(np.asarray(W_proj, dtype=np.float32))
    b_proj = np.ascontiguousarray(np.asarray(b_proj, dtype=np.float32))

    import ml_dtypes

    W_attn_bf = np.ascontiguousarray(
        W_attn.astype(ml_dtypes.bfloat16)
        .reshape(8, 128, 6, 512)
        .transpose(2, 1, 0, 3)
    )
    W_proj_bf = np.ascontiguousarray(W_proj.astype(ml_dtypes.bfloat16))
    b_cols = np.ascontiguousarray(b_attn.reshape(24, 128).T)
    in_maps = [
        {
            "xT": np.ascontiguousarray(x[b].T.astype(ml_dtypes.bfloat16)),
            "amcol": np.ascontiguousarray(am[b].reshape(8, 128).T),
            "bcols": b_cols,
            "W_attn": W_attn_bf,
            "b_attn": b_attn,
            "W_proj": W_proj_bf,
            "b_proj": b_proj,
        }
        for b in range(N_CORES)
    ]
    try:
        res = run_bass_kernel_spmd(nc, in_maps, list(range(N_CORES)))
    except Exception:
        # transient device hiccups (e.g. NRT exec-unit unrecoverable) usually
        # clear on the next attempt
        import time

        time.sleep(15)
        res = run_bass_kernel_spmd(nc, in_maps, list(range(N_CORES)))
    out = np.stack([res.results[b]["out"] for b in range(N_CORES)], axis=0)
    # b_proj is a constant row: applied on host (frees the on-device epilogue)
    out = out + b_proj[None, None, :]
    return out.astype(np.float32)
